# revision 16
# baseline (speedup 1.0000x reference)
"""Trainium2 Bass kernel for MixedIntQuantizedLinear.

Computation (see reference):
  W_dq[o,i] = W_int[o,i] * (scale_i32[o, i//64] / 2^24)
  per-token: amax_t = clip(max|x_t|, 1e-8); s_t = amax_t/127
             q_t = round(x_t / s_t)  (|q| <= 127, round-to-nearest-even)
  y[t,o] = s_t * sum_i q_t[i] * W_dq[o,i] + bias[o]

Sharding over 8 NeuronCores: 2 token-groups (batch halves) x 4
out-feature groups of 1024.  Each core computes y_core [4096, 1024].

Per-core kernel strategy (v6).  The engine/queue assignment is the key:
every engine's instruction FIFO doubles as a DMA issue queue, and each
HWDGE engine (sync, scalar) owns one in-order hardware DMA queue, so
a wait-carrying DMA issue placed in front of an independent one stalls
real transfers.

  - sync engine + qSyncDynamicHW:  x-in tiles (2MB each), oc0 W loads,
    all W transposes (emitted behind the first x tiles so they never
    delay them).
  - scalar engine + qScalarDynamicHW: oc1 W loads (issued first, before
    any scalar compute), qb quantize activations, qT XBAR transposes
    (this keeps the hot qT path off the x-in queue entirely), psum->
    orow epilogue copies.
  - vector (DVE): quant math + 10/16 W-dequant units.
  - gpsimd: y-out DMAs (SWDGE) + 6/16 W-dequant units.

  - W ships as int8 (lossless repack of the int32 carrier); dequant is
    a single fused tensor_tensor: int8 * broadcast fp32 block-scale ->
    bf16, in 16 units of 2 stripes x 1 k-quarter, split across DVE and
    GpSimd so full W is transposed into the resident WT tensors by
    ~tile 2 of the token loop.
  - x tiles [128 tok, 4096]: DVE absmax-reduce; quantize via the fp32
    magic-number trick (x*inv + 1.5*2^23 rounds to int with plain fp32
    RNE); ScalarE subtracts the magic and emits exact-integer bf16;
    one XBAR DMA-transpose produces qT [128, 32, 128].
  - 32 accumulating bf16 matmuls per PSUM tile [128 tok, 512 out];
    epilogue: ScalarE copy with per-partition scale s_t, DVE adds the
    (PE-broadcast) bias row, y DMA issued from the GpSimd queue.
"""

import os
import sys

sys.path.insert(0, "/opt/trn_rl_repo")

import numpy as np

import concourse.bass as bass
import concourse.tile as tile
from concourse import bacc, mybir
from concourse.bass_utils import run_bass_kernel_spmd

P = 128
IN_F = 4096
OUT_F = 4096
TOKENS = 8192          # 4 * 2048
N_CORES = 8
TG = 2                 # token groups
OG = 4                 # out-feature groups
T_CORE = TOKENS // TG  # 4096 tokens per core
O_CORE = OUT_F // OG   # 1024 out features per core
KT = IN_F // P         # 32 contraction tiles
TT = T_CORE // P       # 32 token tiles
OC = O_CORE // 512     # 2 psum chunks of 512
BLOCK = 64
MAGIC = 12582912.0     # 1.5 * 2^23: fp32 round-to-int magic constant
INV_SCALE_SHIFT = 1.0 / (1 << 24)

QF = IN_F // 4         # 1024: quarter-stripe width
QB = QF // BLOCK       # 16 blocks per quarter
QK = KT // 4           # 8 k-tiles per quarter

F32 = mybir.dt.float32
BF16 = mybir.dt.bfloat16
I32 = mybir.dt.int32
I8 = mybir.dt.int8
ACT_COPY = mybir.ActivationFunctionType.Copy

DEPTH = int(os.environ.get("KERNEL_DEPTH", "3"))  # quant pipeline depth

# W-dequant unit = (oc, kq, h): stripes (oc*4+2h, oc*4+2h+1), quarter kq.
# Assignment tuned so each wtq chunk lands just before the PE consumes it
# (PE order: oc0 kq0..3 then oc1 kq0..3, one tile every ~13.8us).  GpSimd
# starts producing at t~2us; DVE shares time with the first three quant
# chains, so it covers the very first chunks and some late oc1 ones.
DVE_UNITS = [(0, 0, 0), (0, 0, 1), (0, 1, 0), (0, 1, 1),
             (1, 2, 0), (1, 2, 1), (1, 3, 0), (1, 3, 1)]
GPS_UNITS = [(0, 2, 0), (0, 2, 1), (0, 3, 0), (0, 3, 1),
             (1, 0, 0), (1, 0, 1), (1, 1, 0), (1, 1, 1)]


def build_kernel():
    nc = bacc.Bacc(None, target_bir_lowering=False, debug=False)

    x_d = nc.dram_tensor("x", [T_CORE, IN_F], F32, kind="ExternalInput")
    w_d = nc.dram_tensor("w", [O_CORE, IN_F], I8, kind="ExternalInput")
    s_d = nc.dram_tensor("s", [O_CORE, BLOCK], I32, kind="ExternalInput")
    b_d = nc.dram_tensor("b", [1, O_CORE], F32, kind="ExternalInput")
    y_d = nc.dram_tensor("y", [T_CORE, O_CORE], F32, kind="ExternalOutput")

    with tile.TileContext(nc) as tc:
        with (
            tc.tile_pool(name="const", bufs=1) as const_pool,
            tc.tile_pool(name="wt", bufs=1) as wt_pool,
            tc.tile_pool(name="psum_y", bufs=6, space="PSUM") as psum_y,
            tc.tile_pool(name="psum_misc", bufs=2, space="PSUM") as psum_misc,
        ):
            # ---- persistent constants ----
            bias_bcast = const_pool.tile([P, O_CORE], F32)
            sc_f32 = const_pool.tile([P, O_CORE // P, BLOCK], F32)

            # WT tensors, one per (oc chunk, k quarter): [128, QK, 512]
            wtq = [[None] * 4 for _ in range(OC)]
            for oc in range(OC):
                for kq in range(4):
                    w = wt_pool.tile([P, QK, 512], BF16, name=f"wt{oc}_{kq}",
                                     tag=f"wt{oc}_{kq}")
                    wtq[oc][kq] = w

            with (
                tc.tile_pool(name="wi8", bufs=2) as wi8_pool,
                tc.tile_pool(name="wbf", bufs=3) as wbf_pool,
                tc.tile_pool(name="xin", bufs=DEPTH) as xin_pool,
                tc.tile_pool(name="small", bufs=2 * DEPTH) as small_pool,
                tc.tile_pool(name="qb", bufs=2) as qb_pool,
                tc.tile_pool(name="qt", bufs=DEPTH) as qt_pool,
                tc.tile_pool(name="orow", bufs=2) as orow_pool,
                tc.tile_pool(name="tmp", bufs=1) as tmp_pool,
            ):
                # ---- bias broadcast row -> [128, O_CORE] via K=1 matmul;
                #      block scales -> fp32 [128, 8, 64] ----
                ones_k1 = tmp_pool.tile([1, P], F32)
                nc.vector.memset(ones_k1[:], 1.0)
                bias_sb = tmp_pool.tile([1, O_CORE], F32)
                nc.sync.dma_start(bias_sb[:], b_d[:])
                sc_i32 = tmp_pool.tile([P, O_CORE // P, BLOCK], I32)
                nc.sync.dma_start(
                    sc_i32[:], s_d.rearrange("(s p) b -> p s b", p=P)
                )
                for oc in range(OC):
                    pb = psum_misc.tile([P, 512], F32, tag="ptr")
                    nc.tensor.matmul(
                        pb[:], ones_k1[:], bias_sb[:, oc * 512:(oc + 1) * 512],
                        start=True, stop=True,
                    )
                    nc.scalar.copy(bias_bcast[:, oc * 512:(oc + 1) * 512],
                                   pb[:])
                nc.vector.tensor_copy(sc_f32[:], sc_i32[:])
                nc.vector.tensor_scalar_mul(sc_f32[:], sc_f32[:],
                                            INV_SCALE_SHIFT)

                # ---- W-prep helpers ----
                wbf_tiles = {}

                def w_load(unit, tag):
                    """DMA one 2-stripe/1-quarter int8 unit (scalar queue:
                    idle at startup, so W transfers start immediately)."""
                    oc, kq, h = unit
                    s0 = oc * 4 + 2 * h
                    wi = wi8_pool.tile([P, 2, QF], I8, tag=tag)
                    nc.scalar.dma_start(
                        wi[:],
                        w_d[s0 * P:(s0 + 2) * P, kq * QF:(kq + 1) * QF]
                        .rearrange("(s p) f -> p s f", p=P),
                    )
                    return wi

                def w_mult(unit, wi, eng, tag):
                    """Fused dequant: int8 * fp32 block scale -> bf16."""
                    oc, kq, h = unit
                    s0 = oc * 4 + 2 * h
                    wb = wbf_pool.tile([P, 2, QF], BF16, tag=tag)
                    eng.tensor_tensor(
                        wb.rearrange("p s (nb j) -> p s nb j", j=BLOCK),
                        wi.rearrange("p s (nb j) -> p s nb j", j=BLOCK),
                        sc_f32[:, s0:s0 + 2, kq * QB:(kq + 1) * QB, None]
                        .to_broadcast((P, 2, QB, BLOCK)),
                        mybir.AluOpType.mult,
                    )
                    wbf_tiles[unit] = wb

                def w_transpose(unit):
                    """XBAR transposes (one per stripe) for a dequant unit,
                    written into the wtq k-tile layout."""
                    oc, kq, h = unit
                    wb = wbf_tiles.pop(unit)
                    for j in range(2):
                        s4 = 2 * h + j
                        nc.sync.dma_start_transpose(
                            wtq[oc][kq][:, :, s4 * P:(s4 + 1) * P],
                            wb[:, j, :],
                        )

                def quant(tt):
                    """Emit the x->qT chain for token tile tt."""
                    xt = xin_pool.tile([P, IN_F], F32, tag="xt")
                    nc.scalar.dma_start(xt[:], x_d[tt * P:(tt + 1) * P, :])
                    amax = small_pool.tile([P, 1], F32, tag="amax")
                    nc.vector.tensor_reduce(
                        amax[:], xt[:], axis=mybir.AxisListType.X,
                        op=mybir.AluOpType.max, apply_absolute_value=True,
                    )
                    s_t = small_pool.tile([P, 1], F32, tag="s_t")
                    nc.vector.tensor_scalar(
                        s_t[:], amax[:], 1e-8, 1.0 / 127.0,
                        op0=mybir.AluOpType.max, op1=mybir.AluOpType.mult,
                    )
                    inv = small_pool.tile([P, 1], F32, tag="inv")
                    nc.vector.reciprocal(inv[:], s_t[:])
                    # x <- x * inv + MAGIC  (fp32; integer part = q + MAGIC)
                    nc.vector.tensor_scalar(
                        xt[:], xt[:], inv[:], MAGIC,
                        op0=mybir.AluOpType.mult, op1=mybir.AluOpType.add,
                    )
                    # q (exact small ints) in bf16
                    qb = qb_pool.tile([P, IN_F], BF16, tag="qb")
                    nc.scalar.activation(qb[:], xt[:], ACT_COPY, bias=-MAGIC)
                    # XBAR transpose -> qT [128(i), KT, 128(t)] on the sync
                    # queue, which carries no other steady-state traffic
                    # (x-in is issued from scalar, y-out from gpsimd).
                    qt = qt_pool.tile([P, KT, P], BF16, tag="qt")
                    nc.sync.dma_start_transpose(qt[:], qb[:])
                    return qt, s_t

                def mms(tt, qt, s_t):
                    """Matmuls + epilogue + output DMA for token tile tt."""
                    orow = orow_pool.tile([P, O_CORE], F32, tag="orow")
                    for oc in range(OC):
                        py = psum_y.tile([P, 512], F32, tag="py")
                        for k in range(KT):
                            nc.tensor.matmul(
                                py[:], qt[:, k, :],
                                wtq[oc][k // QK][:, k % QK, :],
                                start=(k == 0), stop=(k == KT - 1),
                            )
                        nc.scalar.activation(
                            orow[:, oc * 512:(oc + 1) * 512], py[:],
                            ACT_COPY, scale=s_t[:],
                        )
                        nc.vector.tensor_tensor(
                            orow[:, oc * 512:(oc + 1) * 512],
                            orow[:, oc * 512:(oc + 1) * 512],
                            bias_bcast[:, oc * 512:(oc + 1) * 512],
                            mybir.AluOpType.add,
                        )
                    nc.gpsimd.dma_start(y_d[tt * P:(tt + 1) * P, :], orow[:])

                # ---- startup: W loads + dequant interleaved with the
                #      first DEPTH quant chains ----
                # Per-unit emission shape [load(next), mult(cur), wT(cur)]
                # (the v4/v5-proven pattern: the transpose directly follows
                # its producer, so the XBAR never races the dequant).
                def gps_step(i):
                    wi = w_load(GPS_UNITS[i], "wi8_g")
                    w_mult(GPS_UNITS[i], wi, nc.gpsimd, "wbf_g")
                    w_transpose(GPS_UNITS[i])

                def dve_step(i):
                    wi = w_load(DVE_UNITS[i], "wi8_v")
                    w_mult(DVE_UNITS[i], wi, nc.vector, "wbf_v")
                    w_transpose(DVE_UNITS[i])

                pending = {0: quant(0)}              # x0 on scalar
                for i in range(4):
                    gps_step(i)
                    dve_step(i)
                if DEPTH >= 2:
                    pending[1] = quant(1)
                for i in range(4, 7):
                    gps_step(i)
                    dve_step(i)
                gps_step(7)
                if DEPTH >= 3:
                    pending[2] = quant(2)
                dve_step(7)
                for tt in range(3, DEPTH):
                    pending[tt] = quant(tt)

                # ---- main token loop ----
                for tt in range(TT):
                    qt, s_t = pending.pop(tt)
                    if tt + DEPTH < TT:
                        pending[tt + DEPTH] = quant(tt + DEPTH)
                    mms(tt, qt, s_t)

    nc.compile()
    return nc


_NC_CACHE = None


def _get_nc():
    global _NC_CACHE
    if _NC_CACHE is None:
        _NC_CACHE = build_kernel()
    return _NC_CACHE


def kernel(x, W_int, scale_i32, bias, _trace=False, _tmpdir=None):
    nc = _get_nc()
    x2 = np.ascontiguousarray(x, dtype=np.float32).reshape(TOKENS, IN_F)
    W_i8 = np.asarray(W_int).astype(np.int8)          # lossless: [-127,127]
    scale_i32 = np.asarray(scale_i32, dtype=np.int32)
    bias2 = np.asarray(bias, dtype=np.float32).reshape(1, OUT_F)

    in_maps = []
    for c in range(N_CORES):
        tg, og = c // OG, c % OG
        in_maps.append({
            "x": np.ascontiguousarray(x2[tg * T_CORE:(tg + 1) * T_CORE]),
            "w": np.ascontiguousarray(W_i8[og * O_CORE:(og + 1) * O_CORE]),
            "s": np.ascontiguousarray(
                scale_i32[og * O_CORE:(og + 1) * O_CORE]),
            "b": np.ascontiguousarray(bias2[:, og * O_CORE:(og + 1) * O_CORE]),
        })

    res = run_bass_kernel_spmd(
        nc, in_maps, core_ids=list(range(N_CORES)),
        trace=_trace, tmpdir=_tmpdir,
    )
    y = np.empty((TOKENS, OUT_F), dtype=np.float32)
    for c in range(N_CORES):
        tg, og = c // OG, c % OG
        y[tg * T_CORE:(tg + 1) * T_CORE, og * O_CORE:(og + 1) * O_CORE] = \
            res.results[c]["y"]
    out = y.reshape(4, 2048, OUT_F)
    if _trace:
        return out, res
    return out
